# revision 11
# baseline (speedup 1.0000x reference)
"""DeepSeekMoE (E=8, top-2, D=2048, H=1408, T=4096) on 8 TRN2 NeuronCores.

Expert-parallel: core e owns expert e's FFN. The router is tiny
(T x D x E matmul + top-2), so it runs on host in numpy along with the
token dispatch: for each expert, gather its routed tokens' rows,
transpose to [D, C] and pack fp16 in the PE-friendly [p, dt, c] layout,
chunk-major so the first token chunk lands on SBUF within ~2us. Each
core then runs pure FFN GEMMs at full PE rate:
  mm1 (fp16) -> fused SiLU+b1 -> h (fp16, resident)
  mm2 (fp16) -> per-token gate scale -> compact output [C, D] fp16.
Host combines: out[idx_e] += y_e + g_e * b2[e] summed over experts
(all-to-all combine), plus an exact-fp32 host path for any tokens
beyond the C capacity (not hit for balanced loads).
"""

import sys

import numpy as np

sys.path.insert(0, "/opt/trn_rl_repo")

import concourse.bacc as bacc
import concourse.mybir as mybir
import concourse.tile as tile
from concourse.bass_utils import run_bass_kernel_spmd

# Problem shape
T, D, H, E = 4096, 2048, 1408, 8
P = 128
DT, HT = D // P, H // P   # 16, 11
C = 1072                  # per-expert token capacity == max load here
CT = (C + P - 1) // P     # 9 (last tile partial: 48 rows)
CHUNKS = [(0, 512), (512, 512), (1024, 48)]

F32 = mybir.dt.float32
F16 = mybir.dt.float16
AF = mybir.ActivationFunctionType
OP = mybir.AluOpType
ACT_FN = AF.Silu  # sim_test swaps to Sigmoid (Silu not in CoreSim)


def build_nc():
    nc = bacc.Bacc("TRN2", target_bir_lowering=False)

    xgs = [nc.dram_tensor(f"xg{i}", [P, DT * clen], F16, kind="ExternalInput")
           for i, (cs, clen) in enumerate(CHUNKS)]
    w1t = nc.dram_tensor("w1t", [P, HT * DT * P], F16, kind="ExternalInput")
    w2t = nc.dram_tensor("w2t", [P, HT * D], F16, kind="ExternalInput")
    b1c = nc.dram_tensor("b1c", [P, HT], F32, kind="ExternalInput")
    gc = nc.dram_tensor("gc", [P, CT], F32, kind="ExternalInput")
    yo = nc.dram_tensor("yo", [C, D], F16, kind="ExternalOutput")

    with tile.TileContext(nc) as tc:
        with (
            tc.tile_pool(name="res", bufs=1) as res,
            tc.tile_pool(name="io", bufs=2) as io,
            tc.tile_pool(name="ps", bufs=8, space="PSUM") as ps,
        ):
            # sync queue: tiny constants, then token chunks (chunk-major,
            # contiguous). gpsimd queue: w1 in ht-groups, then w2.
            b1c_sb = res.tile([P, HT], F32, name="b1c_sb")
            nc.sync.dma_start(out=b1c_sb[:], in_=b1c[:, :])
            gc_sb = res.tile([P, CT], F32, name="gc_sb")
            nc.sync.dma_start(out=gc_sb[:], in_=gc[:, :])
            xg_sb = [res.tile([P, DT, clen], F16, name=f"xg_sb{i}")
                     for i, (cs, clen) in enumerate(CHUNKS)]
            for i in range(len(CHUNKS)):
                nc.sync.dma_start(out=xg_sb[i][:], in_=xgs[i][:, :])

            w1_sb = res.tile([P, HT, DT, P], F16, name="w1_sb")
            w1v = w1t.rearrange("p (g r) -> p g r", g=4)  # 3+3+3+2 ht-groups
            w1g = w1_sb[:].rearrange("p ht dt q -> p (ht dt q)").rearrange(
                "p (g r) -> p g r", g=4)
            for g in range(4):
                nc.gpsimd.dma_start(out=w1g[:, g], in_=w1v[:, g])
            w2_sb = res.tile([P, HT, D], F16, name="w2_sb")
            nc.gpsimd.dma_start(
                out=w2_sb[:], in_=w2t.rearrange("p (ht d) -> p ht d", ht=HT))

            h_sb = res.tile([P, HT, C], F16, name="h_sb")
            with nc.named_scope("mm1"):
                for i, (cs, clen) in enumerate(CHUNKS):
                    for ht in range(HT):
                        hp = ps.tile([P, 512], F32, tag="mm", name=f"hp_{cs}_{ht}")
                        for dt in range(DT):
                            nc.tensor.matmul(
                                out=hp[:, :clen],
                                lhsT=w1_sb[:, ht, dt, :],
                                rhs=xg_sb[i][:, dt, :],
                                start=(dt == 0), stop=(dt == DT - 1),
                            )
                        nc.scalar.activation(h_sb[:, ht, cs:cs + clen], hp[:, :clen],
                                             ACT_FN, bias=b1c_sb[:, ht:ht + 1])

            with nc.named_scope("mm2"):
                for ct in range(CT):
                    rows = min(P, C - ct * P)
                    ysb = io.tile([P, D], F16, tag="ysb", name=f"ysb_{ct}")
                    for dch in range(4):
                        yp = ps.tile([P, 512], F32, tag="mm", name=f"yp_{ct}_{dch}")
                        for ht in range(HT):
                            nc.tensor.matmul(
                                out=yp[:rows],
                                lhsT=h_sb[:, ht, ct * P:ct * P + rows],
                                rhs=w2_sb[:, ht, dch * 512:(dch + 1) * 512],
                                start=(ht == 0), stop=(ht == HT - 1),
                            )
                        nc.vector.tensor_scalar(ysb[:rows, dch * 512:(dch + 1) * 512],
                                                yp[:rows], gc_sb[:rows, ct:ct + 1],
                                                scalar2=None, op0=OP.mult)
                    nc.sync.dma_start(out=yo[ct * P:ct * P + rows, :],
                                      in_=ysb[:rows])

    nc.compile()
    return nc


_NC_CACHE = {}


def _get_nc():
    if "nc" not in _NC_CACHE:
        _NC_CACHE["nc"] = build_nc()
    return _NC_CACHE["nc"]


def _route(x, gate_w, gate_b, bias):
    scores = x @ gate_w.T + (gate_b + bias)          # [T, E]
    i1 = np.argmax(scores, axis=1)
    ar = np.arange(T)
    s1 = scores[ar, i1]
    sc2 = scores.copy()
    sc2[ar, i1] = -np.inf
    i2 = np.argmax(sc2, axis=1)
    s2 = sc2[ar, i2]
    g1 = 1.0 / (1.0 + np.exp(-(s1 - s2).astype(np.float64)))
    g1 = g1.astype(np.float32)
    g2 = np.float32(1.0) - g1
    return i1, i2, g1, g2


def _prep_core(x16t, w1e, w2e, b1e, idxp, gp):
    """Per-core input map. x16t: [D, T] fp16 (pre-transposed once)."""
    ins = {}
    for i, (cs, clen) in enumerate(CHUNKS):
        blk = x16t[:, idxp[cs:cs + clen]]                      # [D, clen]
        ins[f"xg{i}"] = np.ascontiguousarray(
            blk.reshape(DT, P, clen).transpose(1, 0, 2).reshape(P, DT * clen))
    w1T = w1e.T.astype(np.float16)                             # [D, H]
    ins["w1t"] = np.ascontiguousarray(
        w1T.reshape(DT, P, HT, P).transpose(1, 2, 0, 3).reshape(P, HT * DT * P))
    w2T = w2e.T.astype(np.float16)                             # [H, D]
    ins["w2t"] = np.ascontiguousarray(
        w2T.reshape(HT, P, D).transpose(1, 0, 2).reshape(P, HT * D))
    ins["b1c"] = np.ascontiguousarray(b1e.reshape(HT, P).T)
    gpad = np.zeros(CT * P, dtype=np.float32)
    gpad[:C] = gp
    ins["gc"] = np.ascontiguousarray(gpad.reshape(CT, P).T)
    return ins


def _run(inputs, trace=False):
    x = np.asarray(inputs["x"], dtype=np.float32).reshape(T, D)
    gate_w = np.asarray(inputs["gate_w"], dtype=np.float32)
    gate_b = np.asarray(inputs["gate_b"], dtype=np.float32)
    bias = np.asarray(inputs["bias"], dtype=np.float32)
    w1 = np.asarray(inputs["w1"], dtype=np.float32)
    b1 = np.asarray(inputs["b1"], dtype=np.float32)
    w2 = np.asarray(inputs["w2"], dtype=np.float32)
    b2 = np.asarray(inputs["b2"], dtype=np.float32)

    i1, i2, g1, g2 = _route(x, gate_w, gate_b, bias)

    idx_l, gate_l = [], []
    for e in range(E):
        m1 = np.nonzero(i1 == e)[0]
        m2 = np.nonzero(i2 == e)[0]
        idx_l.append(np.concatenate([m1, m2]))
        gate_l.append(np.concatenate([g1[m1], g2[m2]]))

    x16t = np.ascontiguousarray(x.astype(np.float16).T)        # [D, T]
    in_maps = []
    for e in range(E):
        idx, g = idx_l[e][:C], gate_l[e][:C]
        n = len(idx)
        idxp = np.zeros(C, dtype=np.int64)
        idxp[:n] = idx
        gp = np.zeros(C, dtype=np.float32)
        gp[:n] = g
        in_maps.append(_prep_core(x16t, w1[e], w2[e], b1[e], idxp, gp))

    nc = _get_nc()
    kwargs = {}
    if trace:
        import trace_shim  # noqa: F401
        kwargs = {"trace": True, "trace_cores": list(range(E))}
    res = run_bass_kernel_spmd(nc, in_maps, core_ids=list(range(E)), **kwargs)

    out = np.zeros((T, D), dtype=np.float32)
    for e in range(E):
        idx, g = idx_l[e], gate_l[e]
        n = min(len(idx), C)
        yo = res.results[e]["yo"][:n].astype(np.float32)
        out[idx[:n]] += yo + g[:n, None] * b2[e][None, :]
        if len(idx) > C:  # capacity overflow: exact host path
            xt = x[idx[C:]]
            h = xt @ w1[e].T + b1[e]
            h = h / (1.0 + np.exp(-h))
            out[idx[C:]] += g[C:, None] * (h @ w2[e].T + b2[e])
    return out.reshape(2, T // 2, D), res


def kernel(**inputs) -> np.ndarray:
    out, _ = _run(inputs, trace=False)
    return out
